# revision 21
# baseline (speedup 1.0000x reference)
"""Entmax-alpha Bass kernel for Trainium2, 8-core SPMD.

Problem: out = entmax_bisect(att_scores[4,16,1024,1024], alpha[16]) over last
dim.  Root-solve S(t) = sum_k (s*(x_k - t))_+^p = 1 per row (s = alpha-1,
p = 1/s) by Anderson-Bjorck regula falsi on ln S, then y = (s*(x-t*))^p / S.

E = 1 + KITERS evaluations total (anchor at the bracket's left endpoint plus
KITERS secant iterations; the last one produces the output).  At KITERS=4 the
absmax relative error vs the fp32 reference is ~4.6e-3 (iteration-limited;
fp16 staging adds nothing measurable at this depth).

V2 layout (vs the streaming baseline): x is cast to fp16 on the host and is
fully SBUF-resident (16 supertiles x [128, 4x1024] = 128 KiB/partition), so
HBM traffic is one 16 MiB read plus one 16 MiB fp16 write per core instead of
7 full re-reads.  The clamp+Ln work tile is fp16 (clamp runs on DVE in 4x_2p
perf mode; L needs fp16's mantissa, bf16's 8 bits would cost p*|L|*2^-9 ~ 1%
on the output).  Exp writes a separate bf16 tile: fp16-INPUT reduces crash
the device (NRT_EXEC_UNIT_UNRECOVERABLE / INTERNAL on both tensor_reduce and
tensor_tensor_reduce; bf16 inputs are fine), and bf16 y only costs ~2e-3
relative on the sum.  Row sums/maxes use tensor_tensor_reduce over K/2-halves
(halves the DVE reduce cycles).  Ln/Exp stay on ACT (its cost is
dtype-independent).  The tiny root-state updates are batched across each
chunk of 8 supertiles ([128,32] slices instead of 16x [128,4]) with the two
chunks interleaved per iteration so one chunk's update chain hides under the
other's evaluations.

Sharding: data-parallel over B*H (64 head-blocks) -> 8 blocks per core.
"""

import numpy as np

import concourse.bacc as bacc
import concourse.mybir as mybir
from concourse.tile import TileContext
from concourse.bass_utils import run_bass_kernel_spmd

B, H, Q, K = 4, 16, 1024, 1024
NCORES = 8
BLOCKS = (B * H) // NCORES      # head-blocks per core (8)
import os as _os
R = 4                           # q-subrows per partition per supertile
ST_ROWS = 128 * R               # rows per supertile (512)
N_ST = BLOCKS * Q // ST_ROWS    # supertiles per core (16)
GROUP = int(_os.environ.get("GROUPN", "8"))     # supertiles per chunk
WT_BUFS = int(_os.environ.get("WTBUFS", "4"))   # fp16 work-tile bufs
WY_BUFS = int(_os.environ.get("WYBUFS", "3"))   # bf16 exp-tile bufs
NC = N_ST * R                   # state columns (64)
K_ITERS = int(_os.environ.get("KITERS", "4"))   # secant iterations
CLAMP_POOL = int(_os.environ.get("CPOOL", "0"))  # subrows clamped on gpsimd
EPS = 1e-7

AL = mybir.AluOpType
AF = mybir.ActivationFunctionType
F32 = mybir.dt.float32
F16 = mybir.dt.float16
BF16 = mybir.dt.bfloat16

LAST_RESULT = None              # BassKernelResults of the most recent run


def _build():
    nc = bacc.Bacc(None, target_bir_lowering=False)
    x_in = nc.declare_dram_parameter("x", [BLOCKS * Q, K], F16, isOutput=False)
    cst_in = nc.declare_dram_parameter("cst", [128, 4 * NC], F32,
                                       isOutput=False)
    y_out = nc.declare_dram_parameter("y", [BLOCKS * Q, K], F16, isOutput=True)

    with TileContext(nc) as tc:
        with tc.tile_pool(name="state", bufs=1) as stp, \
             tc.tile_pool(name="xres", bufs=1) as xrp, \
             tc.tile_pool(name="work", bufs=WT_BUFS) as wpp, \
             tc.tile_pool(name="worky", bufs=WY_BUFS) as wyp:
            v = nc.vector

            cst = stp.tile([128, 4 * NC], F32)
            nc.sync.dma_start(cst[:, :], cst_in[:, :])
            c1 = cst[:, 0 * NC:1 * NC]   # 1/s
            c2 = cst[:, 1 * NC:2 * NC]   # ((1/K)^s)/s
            sC = cst[:, 2 * NC:3 * NC]   # s
            pC = cst[:, 3 * NC:4 * NC]   # p = 1/s

            mx = stp.tile([128, NC], F32)
            Pt = stp.tile([128, NC], F32)   # positive-side endpoint (h>=0)
            Nt = stp.tile([128, NC], F32)   # negative-side endpoint (h<=0)
            hp = stp.tile([128, NC], F32)
            hn = stp.tile([128, NC], F32)
            hx = stp.tile([128, NC], F32)
            Sp = stp.tile([128, NC], F32)
            xs = stp.tile([128, NC], F32)   # current evaluation point
            U8 = mybir.dt.uint8
            mpos = stp.tile([128, NC], U8)
            mneg = stp.tile([128, NC], U8)
            ppos = stp.tile([128, NC], U8)  # prev-iter side bits
            pneg = stp.tile([128, NC], U8)
            tm = stp.tile([128, NC], U8)
            t1 = stp.tile([128, NC], F32)
            t2 = stp.tile([128, NC], F32)
            rS = stp.tile([128, NC], F32)

            v.memset(ppos[:, :], 1)
            v.memset(pneg[:, :], 0)

            def x_dram_ap(handle, st):
                r0 = st * ST_ROWS
                return handle[r0:r0 + ST_ROWS, :].rearrange(
                    "(j p) k -> p j k", p=128)

            def sb3(tile_ap):
                return tile_ap.rearrange("p (j k) -> p j k", k=K)

            def ev_clamp(xt, st, t_tile, wt):
                cc = st * R
                for j in range(R):
                    eng = nc.gpsimd if j < CLAMP_POOL else v
                    eng.tensor_scalar(
                        wt[:, j * K:(j + 1) * K], xt[:, j * K:(j + 1) * K],
                        t_tile[:, cc + j:cc + j + 1], EPS,
                        op0=AL.subtract, op1=AL.max)

            def ev_ln(st, wt):
                cc = st * R
                nc.scalar.activation(wt[:, :], wt[:, :], AF.Ln,
                                     scale=sC[:, cc:cc + 1])

            def ev_exp(st, wt, wy):
                cc = st * R
                nc.scalar.activation(wy[:, :], wt[:, :], AF.Exp,
                                     scale=pC[:, cc:cc + 1])

            def ev_sum(st, wy):
                # Row sums via an in-place mult-by-1 tensor_scalar carrying
                # accum_out: TSPtr runs in 4x_2p perf mode on bf16 (267ns per
                # subrow vs 4.3us for a tensor_reduce, which has no perf
                # modes).  tensor_tensor_reduce crashes the device for any
                # 16-bit input; this path is verified on-device.
                cc = st * R
                for j in range(R):
                    v.tensor_scalar(
                        wy[:, j * K:(j + 1) * K], wy[:, j * K:(j + 1) * K],
                        1.0, 0.0, op0=AL.mult, op1=AL.add,
                        accum_out=Sp[:, cc + j:cc + j + 1])

            def do_eval_pair(sts, t_tile, finish):
                """Evaluate a pair of supertiles with the ACT stream
                interleaved (Ln a, Ln b, Exp a, Exp b) so each Exp's
                dependency cleared an entire instruction earlier — avoids the
                ~210ns post-Ln semaphore bubble on ACT per tile."""
                tiles = []
                for st in sts:
                    wt = wpp.tile([128, R * K], F16, name="wt")
                    wy = wyp.tile([128, R * K], BF16, name="wy")
                    ev_clamp(xts[st], st, t_tile, wt)
                    tiles.append((st, wt, wy))
                for st, wt, wy in tiles:
                    ev_ln(st, wt)
                for st, wt, wy in tiles:
                    ev_exp(st, wt, wy)
                for st, wt, wy in tiles:
                    ev_sum(st, wy)
                    finish(st, wt, wy)

            chunks = [list(range(a, min(a + GROUP, N_ST)))
                      for a in range(0, N_ST, GROUP)]
            xts = {}
            # --- load + bracket init + anchor eval at Pt ---
            for chunk in chunks:
                for st in chunk:
                    xt = xrp.tile([128, R * K], F16, name=f"xr{st}")
                    nc.sync.dma_start(sb3(xt[:, :]), x_dram_ap(x_in, st))
                    xts[st] = xt
                    # Row max via value-preserving in-place tensor_scalar
                    # whose accumulator reduces with op1=max (verified
                    # on-device; fp16 input is fine on this path, unlike
                    # tensor_reduce which crashes the device on fp16).
                    for j in range(R):
                        v.tensor_scalar(
                            xt[:, j * K:(j + 1) * K], xt[:, j * K:(j + 1) * K],
                            1.0, -1e30, op0=AL.mult, op1=AL.max,
                            accum_out=mx[:, st * R + j:st * R + j + 1])
                cs = slice(chunk[0] * R, chunk[-1] * R + R)
                v.tensor_tensor(Pt[:, cs], mx[:, cs], c1[:, cs],
                                op=AL.subtract)
                v.tensor_tensor(Nt[:, cs], mx[:, cs], c2[:, cs],
                                op=AL.subtract)
                for a in range(0, len(chunk), 2):
                    do_eval_pair(chunk[a:a + 2], Pt, lambda st, wt, wy: None)
                nc.scalar.activation(hp[:, cs], Sp[:, cs], AF.Ln)
                v.tensor_scalar_mul(hn[:, cs], hp[:, cs], -1.0)

            for it in range(K_ITERS):
                last = it == K_ITERS - 1
                for chunk in chunks:
                    cs = slice(chunk[0] * R, chunk[-1] * R + R)
                    if it > 0:
                        # Anderson-Bjorck update from the previous iteration's
                        # Sp: batched over the whole chunk.
                        nc.scalar.activation(hx[:, cs], Sp[:, cs], AF.Ln)
                        v.tensor_scalar(mpos[:, cs], hx[:, cs], 0.0, None,
                                        op0=AL.is_ge)
                        v.tensor_scalar(mneg[:, cs], hx[:, cs], 0.0, None,
                                        op0=AL.is_lt)
                        # fac = clip(1 - hx/h_same, 0.5, 1) on the stale side
                        v.tensor_tensor(tm[:, cs], mpos[:, cs],
                                        ppos[:, cs], op=AL.bitwise_and)
                        v.tensor_scalar(t1[:, cs], hp[:, cs], 1e-30, None,
                                        op0=AL.max)
                        v.reciprocal(t1[:, cs], t1[:, cs])
                        v.tensor_tensor(t1[:, cs], hx[:, cs], t1[:, cs],
                                        op=AL.mult)
                        v.tensor_scalar(t1[:, cs], t1[:, cs], -1.0, 1.0,
                                        op0=AL.mult, op1=AL.add)
                        v.tensor_scalar(t1[:, cs], t1[:, cs], 0.5, 1.0,
                                        op0=AL.max, op1=AL.min)
                        v.tensor_tensor(t2[:, cs], hn[:, cs], t1[:, cs],
                                        op=AL.mult)
                        v.copy_predicated(hn[:, cs], tm[:, cs], t2[:, cs])
                        v.tensor_tensor(tm[:, cs], mneg[:, cs],
                                        pneg[:, cs], op=AL.bitwise_and)
                        v.tensor_scalar(t1[:, cs], hn[:, cs], -1e-30, None,
                                        op0=AL.min)
                        v.reciprocal(t1[:, cs], t1[:, cs])
                        v.tensor_tensor(t1[:, cs], hx[:, cs], t1[:, cs],
                                        op=AL.mult)
                        v.tensor_scalar(t1[:, cs], t1[:, cs], -1.0, 1.0,
                                        op0=AL.mult, op1=AL.add)
                        v.tensor_scalar(t1[:, cs], t1[:, cs], 0.5, 1.0,
                                        op0=AL.max, op1=AL.min)
                        v.tensor_tensor(t2[:, cs], hp[:, cs], t1[:, cs],
                                        op=AL.mult)
                        v.copy_predicated(hp[:, cs], tm[:, cs], t2[:, cs])
                        v.copy_predicated(hp[:, cs], mpos[:, cs], hx[:, cs])
                        v.copy_predicated(Pt[:, cs], mpos[:, cs], xs[:, cs])
                        v.copy_predicated(hn[:, cs], mneg[:, cs], hx[:, cs])
                        v.copy_predicated(Nt[:, cs], mneg[:, cs], xs[:, cs])
                        v.tensor_copy(ppos[:, cs], mpos[:, cs])
                        v.tensor_copy(pneg[:, cs], mneg[:, cs])
                    # secant point, clipped into the bracket
                    v.tensor_tensor(t1[:, cs], hn[:, cs], hp[:, cs],
                                    op=AL.subtract)
                    v.tensor_scalar_min(t1[:, cs], t1[:, cs], -1e-30)
                    v.reciprocal(t1[:, cs], t1[:, cs])
                    v.tensor_tensor(t2[:, cs], Nt[:, cs], Pt[:, cs],
                                    op=AL.subtract)
                    v.tensor_tensor(t2[:, cs], t2[:, cs], hn[:, cs],
                                    op=AL.mult)
                    v.tensor_tensor(t2[:, cs], t2[:, cs], t1[:, cs],
                                    op=AL.mult)
                    v.tensor_tensor(xs[:, cs], Nt[:, cs], t2[:, cs],
                                    op=AL.subtract)
                    v.tensor_tensor(t1[:, cs], Pt[:, cs], Nt[:, cs],
                                    op=AL.min)
                    v.tensor_tensor(t2[:, cs], Pt[:, cs], Nt[:, cs],
                                    op=AL.max)
                    v.tensor_tensor(xs[:, cs], xs[:, cs], t1[:, cs],
                                    op=AL.max)
                    v.tensor_tensor(xs[:, cs], xs[:, cs], t2[:, cs],
                                    op=AL.min)
                    def finish(st, wt, wy):
                        if not last:
                            return
                        cc = st * R
                        v.reciprocal(rS[:, cc:cc + R], Sp[:, cc:cc + R])
                        for j in range(R):
                            v.tensor_scalar_mul(
                                wt[:, j * K:(j + 1) * K],
                                wy[:, j * K:(j + 1) * K],
                                rS[:, cc + j:cc + j + 1])
                        nc.sync.dma_start(x_dram_ap(y_out, st),
                                          sb3(wt[:, :]))

                    for a in range(0, len(chunk), 2):
                        do_eval_pair(chunk[a:a + 2], xs, finish)
    # Our only ACT functions are Ln and Exp; force the single table set that
    # holds both so no ACT_TABLE_LOAD is ever charged mid-kernel.
    orig_tables = bacc.get_activation_tables

    def _lnexp_only(arch):
        return {k: (v if k == "natural_log_exp_and_others" else set())
                for k, v in orig_tables(arch).items()}

    bacc.get_activation_tables = _lnexp_only
    try:
        nc.finalize()
    finally:
        bacc.get_activation_tables = orig_tables
    return nc


_NC_CACHE = None


def _get_nc():
    global _NC_CACHE
    if _NC_CACHE is None:
        _NC_CACHE = _build()
    return _NC_CACHE


def kernel(att_scores: np.ndarray, alpha: np.ndarray) -> np.ndarray:
    X = np.asarray(att_scores, dtype=np.float32).reshape(B * H, Q, K)
    al = np.asarray(alpha, dtype=np.float64).reshape(H)

    nc = _get_nc()
    in_maps = []
    for c in range(NCORES):
        xc = np.ascontiguousarray(
            X[c * BLOCKS:(c + 1) * BLOCKS].reshape(BLOCKS * Q, K)).astype(
                np.float16)
        cvec = np.zeros((4, NC), np.float64)
        for st in range(N_ST):
            h = (c * BLOCKS + st // (Q // ST_ROWS)) % H
            s = al[h] - 1.0
            cols = slice(st * R, st * R + R)
            # +0.01 margin keeps S(anchor) >= 1 against fp16 rounding of
            # u at the top element, so the bracket stays valid.
            cvec[0, cols] = 1.0 / s + 0.01
            cvec[1, cols] = ((1.0 / K) ** s) / s
            cvec[2, cols] = s
            cvec[3, cols] = 1.0 / s
        cst = np.tile(cvec.reshape(1, 4 * NC).astype(np.float32), (128, 1))
        in_maps.append({"x": xc, "cst": cst})

    res = run_bass_kernel_spmd(nc, in_maps, core_ids=list(range(NCORES)))
    global LAST_RESULT
    LAST_RESULT = res
    outs = [np.asarray(res.results[c]["y"]) for c in range(NCORES)]
    return np.concatenate(outs, axis=0).reshape(B, H, Q, K).astype(np.float32)


# revision 23
# speedup vs baseline: 1.0167x; 1.0167x over previous
"""Entmax-alpha Bass kernel for Trainium2, 8-core SPMD.

Problem: out = entmax_bisect(att_scores[4,16,1024,1024], alpha[16]) over last
dim.  Root-solve S(t) = sum_k (s*(x_k - t))_+^p = 1 per row (s = alpha-1,
p = 1/s) by Anderson-Bjorck regula falsi on ln S, then y = (s*(x-t*))^p / S.

E = 1 + KITERS evaluations total (anchor at the bracket's left endpoint plus
KITERS secant iterations; the last one produces the output).  At KITERS=4 the
absmax relative error vs the fp32 reference is ~4.6e-3 (iteration-limited;
fp16 staging adds nothing measurable at this depth).

V2 layout (vs the streaming baseline): x is cast to fp16 on the host and is
fully SBUF-resident (16 supertiles x [128, 4x1024] = 128 KiB/partition), so
HBM traffic is one 16 MiB read plus one 16 MiB fp16 write per core instead of
7 full re-reads.  The clamp+Ln work tile is fp16 (clamp runs on DVE in 4x_2p
perf mode; L needs fp16's mantissa, bf16's 8 bits would cost p*|L|*2^-9 ~ 1%
on the output).  Exp writes a separate bf16 tile: fp16-INPUT reduces crash
the device (NRT_EXEC_UNIT_UNRECOVERABLE / INTERNAL on both tensor_reduce and
tensor_tensor_reduce; bf16 inputs are fine), and bf16 y only costs ~2e-3
relative on the sum.  Row sums/maxes use tensor_tensor_reduce over K/2-halves
(halves the DVE reduce cycles).  Ln/Exp stay on ACT (its cost is
dtype-independent).  The tiny root-state updates are batched across each
chunk of 8 supertiles ([128,32] slices instead of 16x [128,4]) with the two
chunks interleaved per iteration so one chunk's update chain hides under the
other's evaluations.

Sharding: data-parallel over B*H (64 head-blocks) -> 8 blocks per core.
"""

import numpy as np

import concourse.bacc as bacc
import concourse.mybir as mybir
from concourse.tile import TileContext
from concourse.bass_utils import run_bass_kernel_spmd

B, H, Q, K = 4, 16, 1024, 1024
NCORES = 8
BLOCKS = (B * H) // NCORES      # head-blocks per core (8)
import os as _os
R = 4                           # q-subrows per partition per supertile
ST_ROWS = 128 * R               # rows per supertile (512)
N_ST = BLOCKS * Q // ST_ROWS    # supertiles per core (16)
GROUP = int(_os.environ.get("GROUPN", "8"))     # supertiles per chunk
WT_BUFS = int(_os.environ.get("WTBUFS", "4"))   # fp16 work-tile bufs
WY_BUFS = int(_os.environ.get("WYBUFS", "3"))   # bf16 exp-tile bufs
NC = N_ST * R                   # state columns (64)
K_ITERS = int(_os.environ.get("KITERS", "4"))   # secant iterations
CLAMP_POOL = int(_os.environ.get("CPOOL", "0"))  # subrows clamped on gpsimd
PAIR = int(_os.environ.get("PAIR", "1"))         # supertiles emitted together
EPS = 1e-7

AL = mybir.AluOpType
AF = mybir.ActivationFunctionType
F32 = mybir.dt.float32
F16 = mybir.dt.float16
BF16 = mybir.dt.bfloat16

LAST_RESULT = None              # BassKernelResults of the most recent run


def _build():
    nc = bacc.Bacc(None, target_bir_lowering=False)
    x_in = nc.declare_dram_parameter("x", [BLOCKS * Q, K], F16, isOutput=False)
    cst_in = nc.declare_dram_parameter("cst", [128, 4 * NC], F32,
                                       isOutput=False)
    y_out = nc.declare_dram_parameter("y", [BLOCKS * Q, K], F16, isOutput=True)

    with TileContext(nc) as tc:
        with tc.tile_pool(name="state", bufs=1) as stp, \
             tc.tile_pool(name="xres", bufs=1) as xrp, \
             tc.tile_pool(name="work", bufs=WT_BUFS) as wpp, \
             tc.tile_pool(name="worky", bufs=WY_BUFS) as wyp:
            v = nc.vector

            cst = stp.tile([128, 4 * NC], F32)
            nc.sync.dma_start(cst[:, :], cst_in[:, :])
            c1 = cst[:, 0 * NC:1 * NC]   # 1/s
            c2 = cst[:, 1 * NC:2 * NC]   # ((1/K)^s)/s
            sC = cst[:, 2 * NC:3 * NC]   # s
            pC = cst[:, 3 * NC:4 * NC]   # p = 1/s

            mx = stp.tile([128, NC], F32)
            Pt = stp.tile([128, NC], F32)   # positive-side endpoint (h>=0)
            Nt = stp.tile([128, NC], F32)   # negative-side endpoint (h<=0)
            hp = stp.tile([128, NC], F32)
            hn = stp.tile([128, NC], F32)
            hx = stp.tile([128, NC], F32)
            Sp = stp.tile([128, NC], F32)
            xs = stp.tile([128, NC], F32)   # current evaluation point
            U8 = mybir.dt.uint8
            mpos = stp.tile([128, NC], U8)
            mneg = stp.tile([128, NC], U8)
            ppos = stp.tile([128, NC], U8)  # prev-iter side bits
            pneg = stp.tile([128, NC], U8)
            tm = stp.tile([128, NC], U8)
            t1 = stp.tile([128, NC], F32)
            t2 = stp.tile([128, NC], F32)
            rS = stp.tile([128, NC], F32)

            v.memset(ppos[:, :], 1)
            v.memset(pneg[:, :], 0)

            def x_dram_ap(handle, st):
                r0 = st * ST_ROWS
                return handle[r0:r0 + ST_ROWS, :].rearrange(
                    "(j p) k -> p j k", p=128)

            def sb3(tile_ap):
                return tile_ap.rearrange("p (j k) -> p j k", k=K)

            def ev_clamp(xt, st, t_tile, wt):
                cc = st * R
                for j in range(R):
                    eng = nc.gpsimd if j < CLAMP_POOL else v
                    eng.tensor_scalar(
                        wt[:, j * K:(j + 1) * K], xt[:, j * K:(j + 1) * K],
                        t_tile[:, cc + j:cc + j + 1], EPS,
                        op0=AL.subtract, op1=AL.max)

            def ev_ln(st, wt):
                cc = st * R
                nc.scalar.activation(wt[:, :], wt[:, :], AF.Ln,
                                     scale=sC[:, cc:cc + 1])

            def ev_exp(st, wt, wy):
                cc = st * R
                nc.scalar.activation(wy[:, :], wt[:, :], AF.Exp,
                                     scale=pC[:, cc:cc + 1])

            def ev_sum(st, wy):
                # Row sums via an in-place mult-by-1 tensor_scalar carrying
                # accum_out: TSPtr runs in 4x_2p perf mode on bf16 (267ns per
                # subrow vs 4.3us for a tensor_reduce, which has no perf
                # modes).  tensor_tensor_reduce crashes the device for any
                # 16-bit input; this path is verified on-device.
                cc = st * R
                for j in range(R):
                    v.tensor_scalar(
                        wy[:, j * K:(j + 1) * K], wy[:, j * K:(j + 1) * K],
                        1.0, 0.0, op0=AL.mult, op1=AL.add,
                        accum_out=Sp[:, cc + j:cc + j + 1])

            def do_eval_pair(sts, t_tile, finish):
                """Evaluate a pair of supertiles with the ACT stream
                interleaved (Ln a, Ln b, Exp a, Exp b) so each Exp's
                dependency cleared an entire instruction earlier — avoids the
                ~210ns post-Ln semaphore bubble on ACT per tile."""
                tiles = []
                for st in sts:
                    wt = wpp.tile([128, R * K], F16, name="wt")
                    wy = wyp.tile([128, R * K], BF16, name="wy")
                    ev_clamp(xts[st], st, t_tile, wt)
                    tiles.append((st, wt, wy))
                for st, wt, wy in tiles:
                    ev_ln(st, wt)
                for st, wt, wy in tiles:
                    ev_exp(st, wt, wy)
                for st, wt, wy in tiles:
                    ev_sum(st, wy)
                    finish(st, wt, wy)

            chunks = [list(range(a, min(a + GROUP, N_ST)))
                      for a in range(0, N_ST, GROUP)]
            xts = {}
            # --- load + bracket init + anchor eval at Pt ---
            for chunk in chunks:
                for st in chunk:
                    xt = xrp.tile([128, R * K], F16, name=f"xr{st}")
                    nc.sync.dma_start(sb3(xt[:, :]), x_dram_ap(x_in, st))
                    xts[st] = xt
                    # Row max via value-preserving in-place tensor_scalar
                    # whose accumulator reduces with op1=max (verified
                    # on-device; fp16 input is fine on this path, unlike
                    # tensor_reduce which crashes the device on fp16).
                    for j in range(R):
                        v.tensor_scalar(
                            xt[:, j * K:(j + 1) * K], xt[:, j * K:(j + 1) * K],
                            1.0, -1e30, op0=AL.mult, op1=AL.max,
                            accum_out=mx[:, st * R + j:st * R + j + 1])
                cs = slice(chunk[0] * R, chunk[-1] * R + R)
                v.tensor_tensor(Pt[:, cs], mx[:, cs], c1[:, cs],
                                op=AL.subtract)
                v.tensor_tensor(Nt[:, cs], mx[:, cs], c2[:, cs],
                                op=AL.subtract)
                for a in range(0, len(chunk), PAIR):
                    do_eval_pair(chunk[a:a + PAIR], Pt, lambda st, wt, wy: None)
                nc.scalar.activation(hp[:, cs], Sp[:, cs], AF.Ln)
                v.tensor_scalar_mul(hn[:, cs], hp[:, cs], -1.0)

            for it in range(K_ITERS):
                last = it == K_ITERS - 1
                for chunk in chunks:
                    cs = slice(chunk[0] * R, chunk[-1] * R + R)
                    if it > 0:
                        # Anderson-Bjorck update from the previous iteration's
                        # Sp: batched over the whole chunk.
                        nc.scalar.activation(hx[:, cs], Sp[:, cs], AF.Ln)
                        v.tensor_scalar(mpos[:, cs], hx[:, cs], 0.0, None,
                                        op0=AL.is_ge)
                        v.tensor_scalar(mneg[:, cs], hx[:, cs], 0.0, None,
                                        op0=AL.is_lt)
                        # fac = clip(1 - hx/h_same, 0.5, 1) on the stale side
                        v.tensor_tensor(tm[:, cs], mpos[:, cs],
                                        ppos[:, cs], op=AL.bitwise_and)
                        v.tensor_scalar(t1[:, cs], hp[:, cs], 1e-30, None,
                                        op0=AL.max)
                        v.reciprocal(t1[:, cs], t1[:, cs])
                        v.tensor_tensor(t1[:, cs], hx[:, cs], t1[:, cs],
                                        op=AL.mult)
                        v.tensor_scalar(t1[:, cs], t1[:, cs], -1.0, 1.0,
                                        op0=AL.mult, op1=AL.add)
                        v.tensor_scalar(t1[:, cs], t1[:, cs], 0.5, 1.0,
                                        op0=AL.max, op1=AL.min)
                        v.tensor_tensor(t2[:, cs], hn[:, cs], t1[:, cs],
                                        op=AL.mult)
                        v.copy_predicated(hn[:, cs], tm[:, cs], t2[:, cs])
                        v.tensor_tensor(tm[:, cs], mneg[:, cs],
                                        pneg[:, cs], op=AL.bitwise_and)
                        v.tensor_scalar(t1[:, cs], hn[:, cs], -1e-30, None,
                                        op0=AL.min)
                        v.reciprocal(t1[:, cs], t1[:, cs])
                        v.tensor_tensor(t1[:, cs], hx[:, cs], t1[:, cs],
                                        op=AL.mult)
                        v.tensor_scalar(t1[:, cs], t1[:, cs], -1.0, 1.0,
                                        op0=AL.mult, op1=AL.add)
                        v.tensor_scalar(t1[:, cs], t1[:, cs], 0.5, 1.0,
                                        op0=AL.max, op1=AL.min)
                        v.tensor_tensor(t2[:, cs], hp[:, cs], t1[:, cs],
                                        op=AL.mult)
                        v.copy_predicated(hp[:, cs], tm[:, cs], t2[:, cs])
                        v.copy_predicated(hp[:, cs], mpos[:, cs], hx[:, cs])
                        v.copy_predicated(Pt[:, cs], mpos[:, cs], xs[:, cs])
                        v.copy_predicated(hn[:, cs], mneg[:, cs], hx[:, cs])
                        v.copy_predicated(Nt[:, cs], mneg[:, cs], xs[:, cs])
                        v.tensor_copy(ppos[:, cs], mpos[:, cs])
                        v.tensor_copy(pneg[:, cs], mneg[:, cs])
                    # secant point, clipped into the bracket
                    v.tensor_tensor(t1[:, cs], hn[:, cs], hp[:, cs],
                                    op=AL.subtract)
                    v.tensor_scalar_min(t1[:, cs], t1[:, cs], -1e-30)
                    v.reciprocal(t1[:, cs], t1[:, cs])
                    v.tensor_tensor(t2[:, cs], Nt[:, cs], Pt[:, cs],
                                    op=AL.subtract)
                    v.tensor_tensor(t2[:, cs], t2[:, cs], hn[:, cs],
                                    op=AL.mult)
                    v.tensor_tensor(t2[:, cs], t2[:, cs], t1[:, cs],
                                    op=AL.mult)
                    v.tensor_tensor(xs[:, cs], Nt[:, cs], t2[:, cs],
                                    op=AL.subtract)
                    v.tensor_tensor(t1[:, cs], Pt[:, cs], Nt[:, cs],
                                    op=AL.min)
                    v.tensor_tensor(t2[:, cs], Pt[:, cs], Nt[:, cs],
                                    op=AL.max)
                    v.tensor_tensor(xs[:, cs], xs[:, cs], t1[:, cs],
                                    op=AL.max)
                    v.tensor_tensor(xs[:, cs], xs[:, cs], t2[:, cs],
                                    op=AL.min)
                    def finish(st, wt, wy):
                        if not last:
                            return
                        cc = st * R
                        v.reciprocal(rS[:, cc:cc + R], Sp[:, cc:cc + R])
                        for j in range(R):
                            v.tensor_scalar_mul(
                                wt[:, j * K:(j + 1) * K],
                                wy[:, j * K:(j + 1) * K],
                                rS[:, cc + j:cc + j + 1])
                        nc.sync.dma_start(x_dram_ap(y_out, st),
                                          sb3(wt[:, :]))

                    for a in range(0, len(chunk), PAIR):
                        do_eval_pair(chunk[a:a + PAIR], xs, finish)
    # Our only ACT functions are Ln and Exp; force the single table set that
    # holds both so no ACT_TABLE_LOAD is ever charged mid-kernel.
    orig_tables = bacc.get_activation_tables

    def _lnexp_only(arch):
        return {k: (v if k == "natural_log_exp_and_others" else set())
                for k, v in orig_tables(arch).items()}

    bacc.get_activation_tables = _lnexp_only
    try:
        nc.finalize()
    finally:
        bacc.get_activation_tables = orig_tables
    return nc


_NC_CACHE = None


def _get_nc():
    global _NC_CACHE
    if _NC_CACHE is None:
        _NC_CACHE = _build()
    return _NC_CACHE


def kernel(att_scores: np.ndarray, alpha: np.ndarray) -> np.ndarray:
    X = np.asarray(att_scores, dtype=np.float32).reshape(B * H, Q, K)
    al = np.asarray(alpha, dtype=np.float64).reshape(H)

    nc = _get_nc()
    in_maps = []
    for c in range(NCORES):
        xc = np.ascontiguousarray(
            X[c * BLOCKS:(c + 1) * BLOCKS].reshape(BLOCKS * Q, K)).astype(
                np.float16)
        cvec = np.zeros((4, NC), np.float64)
        for st in range(N_ST):
            h = (c * BLOCKS + st // (Q // ST_ROWS)) % H
            s = al[h] - 1.0
            cols = slice(st * R, st * R + R)
            # +0.01 margin keeps S(anchor) >= 1 against fp16 rounding of
            # u at the top element, so the bracket stays valid.
            cvec[0, cols] = 1.0 / s + 0.01
            cvec[1, cols] = ((1.0 / K) ** s) / s
            cvec[2, cols] = s
            cvec[3, cols] = 1.0 / s
        cst = np.tile(cvec.reshape(1, 4 * NC).astype(np.float32), (128, 1))
        in_maps.append({"x": xc, "cst": cst})

    res = run_bass_kernel_spmd(nc, in_maps, core_ids=list(range(NCORES)))
    global LAST_RESULT
    LAST_RESULT = res
    outs = [np.asarray(res.results[c]["y"]) for c in range(NCORES)]
    return np.concatenate(outs, axis=0).reshape(B, H, Q, K).astype(np.float32)
